# revision 1
# baseline (speedup 1.0000x reference)
"""Bass/Tile TRN2 kernel for nn_BatchAdditiveAttention.

Math (per batch, per node n):
    f_proj      = feature @ w1.T                        # (n, 128)
    t_proj[r]   = temb[:, r] @ w2.T                     # (n, 4, 128)
    q[r]        = tanh(f_proj + t_proj[r])              # (n, 4, 128)
    score[r]    = q[r] @ m                              # (n, 4)
    beta        = softmax_r(score)                      # (n, 4)
    out         = sum_r beta[r] * temb[:, r]            # (n, 256)

Sharding: data-parallel over bs=8, one batch per NeuronCore.

On-chip layout: the d-contraction of the projections requires d on
partitions, so per-node-tile the inputs are PE-transposed (bf16,
cast during the SWDGE DMA load) and the projections accumulate in
PSUM as q_pre[e=128, n].  Scores use q as the stationary matmul
operand against the small m column, giving score[n, r] with n on
partitions, so the r-softmax is a cheap free-dim reduce + one Exp
with fused bias (-max) and fused accumulated sum.  The fused output
uses diag(exp) as stationary weights against the natural-layout temb
tile, accumulating sum_r exp[r]*temb[r] in PSUM; the PSUM->SBUF copy
applies the 1/sum normalization as a per-partition tensor_scalar.
"""

import os
from contextlib import ExitStack

import numpy as np
import ml_dtypes

import concourse.bass as bass
import concourse.tile as tile
from concourse import bacc, mybir

BS = 8
N_NODES = 20000
D = 256
R = 4
D2 = 128
NT = 512  # nodes per tile
PB = 128  # nodes per sub-tile (partition block)

BF16 = mybir.dt.bfloat16
F32 = mybir.dt.float32
AX = mybir.AxisListType
ALU = mybir.AluOpType
ACTF = mybir.ActivationFunctionType


def _sub_blocks(nt):
    """Split a node-tile of nt nodes into partition blocks of <=128."""
    blocks = []
    off = 0
    while off < nt:
        blocks.append((off // PB, min(PB, nt - off)))
        off += PB
    return blocks


DEFAULT_OPTS = dict(
    act_copies=3,   # how many of the 10 transpose copy-backs go to ACT
    tps_bufs=3,
    sc_bufs=1,
    fp_bufs=2,
    io_bufs=4,
    tp_bufs=8,
    q_bufs=6,
    batch2=False,   # pair-batch F loads + output stores (measured neutral/worse)
    mscore=False,   # scores via m-stationary rows + tiny PE transpose
    gpsimd_diag=False,  # build diag(exp) tiles on GPSIMD instead of DVE
)


def build_kernel_body(ctx, tc, n_nodes, aps, opts=None, time_reps=None):
    o = dict(DEFAULT_OPTS, **(opts or {}))
    nc = tc.nc
    feat, temb, w1t, w2t, mcol, eye, out = aps

    const = ctx.enter_context(tc.tile_pool(name="const", bufs=1))
    fio = ctx.enter_context(tc.tile_pool(name="fio", bufs=o["io_bufs"]))
    tio = ctx.enter_context(tc.tile_pool(name="tio", bufs=o["io_bufs"]))
    ftp = ctx.enter_context(tc.tile_pool(name="ftp", bufs=o["tp_bufs"]))
    ttp = ctx.enter_context(tc.tile_pool(name="ttp", bufs=o["tp_bufs"]))
    qpool = ctx.enter_context(tc.tile_pool(name="qpool", bufs=o["q_bufs"]))
    small = ctx.enter_context(tc.tile_pool(name="small", bufs=4))
    opool = ctx.enter_context(tc.tile_pool(name="opool", bufs=3))
    tps_bufs = o["tps_bufs"] - 1 if o["mscore"] else o["tps_bufs"]
    tpsum = ctx.enter_context(tc.tile_pool(name="tpsum", bufs=tps_bufs, space="PSUM"))
    qpsum = ctx.enter_context(tc.tile_pool(name="qpsum", bufs=2, space="PSUM"))
    spsum = ctx.enter_context(tc.tile_pool(name="spsum", bufs=o["sc_bufs"], space="PSUM"))
    fpsum = ctx.enter_context(tc.tile_pool(name="fpsum", bufs=o["fp_bufs"], space="PSUM"))
    mpsum = (
        ctx.enter_context(tc.tile_pool(name="mpsum", bufs=1, space="PSUM"))
        if o["mscore"]
        else None
    )

    # Constants: weight chunks [d-chunk][d=128, e=128], m column, identity.
    w1sb = const.tile([128, 2, D2], BF16)
    w2sb = const.tile([128, 2, D2], BF16)
    msb = const.tile([128, 1], BF16)
    eyesb = const.tile([128, 128], BF16)
    for c in range(2):
        nc.sync.dma_start(out=w1sb[:, c, :], in_=w1t[c])
        nc.sync.dma_start(out=w2sb[:, c, :], in_=w2t[c])
    nc.sync.dma_start(out=msb[:], in_=mcol[:])
    nc.sync.dma_start(out=eyesb[:], in_=eye[:])
    eye4 = None
    if o["mscore"]:
        # tiny f32 identity for the [4, n] -> [n, 4] score transposes
        eye4 = const.tile([4, 4], F32)
        nc.vector.tensor_copy(eye4[:], eyesb[0:4, 0:4])

    rep_cm = tc.For_i(0, time_reps, 1) if time_reps else None
    if rep_cm is not None:
        ctx.enter_context(rep_cm)
    PAIR = 2 * NT
    fn_pair = [None]
    osb_pair = [None]
    for t0 in range(0, n_nodes, NT):
        nt = min(NT, n_nodes - t0)
        blocks = _sub_blocks(nt)
        na = len(blocks)
        p = min(PB, nt)  # partition width of the natural-layout tiles
        ti = t0 // NT
        bt0 = (ti - ti % 2) * NT
        bnt = min(PAIR, n_nodes - bt0)
        # batch F-load/store over the pair only when it is 2 full tiles
        batched = bnt == PAIR and o["batch2"]
        pair_first = (ti % 2 == 0) or not batched
        bna = (bnt + PB - 1) // PB
        bp = min(PB, bnt)
        ao = 0 if (ti % 2 == 0) else 4  # a-offset of this tile inside the pair

        # ---- load natural tiles (f32 -> bf16 cast in SWDGE DMA) ----
        if batched:
            if ti % 2 == 0:
                fn_pair[0] = fio.tile([128, 8, D], BF16, tag="fn", name="fn")
                nc.gpsimd.dma_start(
                    out=fn_pair[0][0:bp, 0:bna, :],
                    in_=feat[bt0 : bt0 + bnt].rearrange("(a p) d -> p a d", p=bp),
                )
                osb_pair[0] = opool.tile([128, 8, D], F32, tag="osb", name="osb")
            fn = fn_pair[0][:, ao : ao + 4, :]
            osb = osb_pair[0][:, ao : ao + 4, :]
        else:
            fn_pair[0] = fio.tile([128, 8, D], BF16, tag="fn", name="fn")
            nc.gpsimd.dma_start(
                out=fn_pair[0][0:p, 0:na, :],
                in_=feat[t0 : t0 + nt].rearrange("(a p) d -> p a d", p=p),
            )
            osb_pair[0] = opool.tile([128, 8, D], F32, tag="osb", name="osb")
            fn = fn_pair[0][:, 0:4, :]
            osb = osb_pair[0][:, 0:4, :]
        tn = tio.tile([128, 4, R, D], BF16, tag="tn")
        nc.gpsimd.dma_start(
            out=tn[0:p, 0:na, :, :],
            in_=temb[t0 : t0 + nt].rearrange("(a p) r d -> p a r d", p=p),
        )

        # ---- transpose all F/T blocks: 10 groups of [d=128, n<=512] ----
        # group list: ("f", c) x2 then ("t", r, c) x8
        n_copy = 0

        def transpose_group(src_slice_fn, pool, tag):
            nonlocal n_copy
            ps = tpsum.tile([128, NT], BF16, tag="tps")
            for a, ns in blocks:
                nc.tensor.transpose(
                    ps[:, a * PB : a * PB + ns],
                    src_slice_fn(a, ns),
                    eyesb[0:ns, 0:ns],
                )
            t = pool.tile([128, NT], BF16, tag=tag)
            if n_copy % 10 < o["act_copies"]:
                nc.scalar.copy(t[:, 0:nt], ps[:, 0:nt])
            else:
                nc.vector.tensor_copy(t[:, 0:nt], ps[:, 0:nt])
            n_copy += 1
            return t

        ft = [
            transpose_group(
                lambda a, ns, c=c: fn[0:ns, a, c * 128 : (c + 1) * 128], ftp, "ft"
            )
            for c in range(2)
        ]
        tt = {
            (r, c): transpose_group(
                lambda a, ns, r=r, c=c: tn[0:ns, a, r, c * 128 : (c + 1) * 128],
                ttp, "tt",
            )
            for r in range(R)
            for c in range(2)
        }

        scores = spsum.tile([128, 4 * R], F32, tag="sc")
        mrows = (
            mpsum.tile([128, NT], F32, tag="msc", name="mrows")
            if o["mscore"]
            else None
        )
        qsb = []
        for r in range(R):
            # ---- q_pre[e, n] = w1.T.T@ft + w2.T.T@tt (PSUM accumulate) ----
            qp = qpsum.tile([128, NT], F32, tag="qp")
            nc.tensor.matmul(qp[:, 0:nt], w1sb[:, 0, :], ft[0][:, 0:nt],
                             start=True, stop=False)
            nc.tensor.matmul(qp[:, 0:nt], w1sb[:, 1, :], ft[1][:, 0:nt],
                             start=False, stop=False)
            nc.tensor.matmul(qp[:, 0:nt], w2sb[:, 0, :], tt[r, 0][:, 0:nt],
                             start=False, stop=False)
            nc.tensor.matmul(qp[:, 0:nt], w2sb[:, 1, :], tt[r, 1][:, 0:nt],
                             start=False, stop=True)

            # ---- q = tanh(q_pre), bf16 for the score matmuls ----
            q = qpool.tile([128, NT], BF16, tag="q")
            nc.scalar.activation(q[:, 0:nt], qp[:, 0:nt], ACTF.Tanh)
            qsb.append(q)

            if o["mscore"]:
                # score row [1, n] at psum partition 32r (m stationary)
                nc.tensor.matmul(
                    mrows[32 * r : 32 * r + 1, 0:nt],
                    msb[:, 0:1],
                    q[:, 0:nt],
                    start=True, stop=True,
                    tile_position=(0, 32 * r),
                )
            else:
                # score[n, r] per block: q block stationary, m moving
                for a, ns in blocks:
                    nc.tensor.matmul(
                        scores[0:ns, a * R + r : a * R + r + 1],
                        q[:, a * PB : a * PB + ns],
                        msb[:, 0:1],
                        start=True, stop=True,
                    )

        if o["mscore"]:
            # gather the 4 score rows to SBUF, then tiny PE transposes
            # into scores[n, 4] per block
            sc4 = small.tile([4, NT], F32, tag="sc4")
            nc.vector.tensor_copy(sc4[:, 0:nt], mrows[0:128:32, 0:nt])
            for a, ns in blocks:
                nc.tensor.transpose(
                    scores[0:ns, a * R : (a + 1) * R],
                    sc4[:, a * PB : a * PB + ns],
                    eye4[:],
                )

        # ---- softmax over r + fused output, per block ----
        for a, ns in blocks:
            sc = scores[0:ns, a * R : (a + 1) * R]
            negmax = small.tile([128, 1], F32, tag="negmax")
            nc.vector.tensor_reduce(negmax[0:ns], sc, AX.X, ALU.max, negate=True)
            expo = small.tile([128, R], F32, tag="expo")
            sume = small.tile([128, 1], F32, tag="sume")
            nc.scalar.activation(expo[0:ns], sc, ACTF.Exp,
                                 bias=negmax[0:ns], accum_out=sume[0:ns])
            inv = small.tile([128, 1], F32, tag="inv")
            nc.vector.reciprocal(inv[0:ns], sume[0:ns])

            fp = fpsum.tile([128, D], F32, tag="fp")
            diag_eng = nc.gpsimd if o["gpsimd_diag"] else nc.vector
            for r in range(R):
                diag = small.tile([128, 128], BF16, tag="diag")
                diag_eng.tensor_scalar_mul(
                    diag[0:ns, 0:ns], eyesb[0:ns, 0:ns], expo[0:ns, r : r + 1]
                )
                nc.tensor.matmul(fp[0:ns, :], diag[0:ns, 0:ns],
                                 tn[0:ns, a, r, :],
                                 start=(r == 0), stop=(r == R - 1))
            nc.vector.tensor_scalar_mul(osb[0:ns, a, :], fp[0:ns, :],
                                        inv[0:ns, 0:1])

        if batched and ti % 2 == 1:
            nc.sync.dma_start(
                out=out[bt0 : bt0 + bnt].rearrange("(a p) d -> p a d", p=bp),
                in_=osb_pair[0][0:bp, 0:bna, :],
            )
        elif not batched:
            nc.sync.dma_start(
                out=out[t0 : t0 + nt].rearrange("(a p) d -> p a d", p=p),
                in_=osb_pair[0][0:p, 0:na, :],
            )


def build_kernel_body_pt(ctx, tc, n_nodes, aps, opts=None, time_reps=None):
    """Pretransposed variant: host supplies bf16 natural temb + bf16
    transposed temb/feature; no on-chip transposes or copy-backs."""
    o = dict(DEFAULT_OPTS, **(opts or {}))
    nc = tc.nc
    temb_n, temb_t, feat_t, w1t, w2t, mcol, eye, out = aps

    const = ctx.enter_context(tc.tile_pool(name="const", bufs=1))
    tio = ctx.enter_context(tc.tile_pool(name="tio", bufs=o["io_bufs"]))
    ttio = ctx.enter_context(tc.tile_pool(name="ttio", bufs=o["io_bufs"]))
    ftio = ctx.enter_context(tc.tile_pool(name="ftio", bufs=o["io_bufs"]))
    qpool = ctx.enter_context(tc.tile_pool(name="qpool", bufs=o["q_bufs"]))
    small = ctx.enter_context(tc.tile_pool(name="small", bufs=4))
    opool = ctx.enter_context(tc.tile_pool(name="opool", bufs=3))
    qpsum = ctx.enter_context(tc.tile_pool(name="qpsum", bufs=3, space="PSUM"))
    spsum = ctx.enter_context(tc.tile_pool(name="spsum", bufs=2, space="PSUM"))
    fpsum = ctx.enter_context(tc.tile_pool(name="fpsum", bufs=3, space="PSUM"))

    w1sb = const.tile([128, 2, D2], BF16)
    w2sb = const.tile([128, 2, D2], BF16)
    msb = const.tile([128, 1], BF16)
    eyesb = const.tile([128, 128], BF16)
    for c in range(2):
        nc.sync.dma_start(out=w1sb[:, c, :], in_=w1t[c])
        nc.sync.dma_start(out=w2sb[:, c, :], in_=w2t[c])
    nc.sync.dma_start(out=msb[:], in_=mcol[:])
    nc.sync.dma_start(out=eyesb[:], in_=eye[:])

    rep_cm = tc.For_i(0, time_reps, 1) if time_reps else None
    if rep_cm is not None:
        ctx.enter_context(rep_cm)
    for t0 in range(0, n_nodes, NT):
        nt = min(NT, n_nodes - t0)
        blocks = _sub_blocks(nt)
        na = len(blocks)
        p = min(PB, nt)

        tn = tio.tile([128, 4, R, D], BF16, tag="tn")
        nc.gpsimd.dma_start(
            out=tn[0:p, 0:na, :, :],
            in_=temb_n[t0 : t0 + nt].rearrange("(a p) r d -> p a r d", p=p),
        )
        tt = ttio.tile([128, R, 2, NT], BF16, tag="tt")
        nc.gpsimd.dma_start(
            out=tt[:, :, :, 0:nt],
            in_=temb_t[:, :, :, t0 : t0 + nt].rearrange("r c p n -> p r c n"),
        )
        ft = ftio.tile([128, 2, NT], BF16, tag="ft")
        nc.gpsimd.dma_start(
            out=ft[:, :, 0:nt],
            in_=feat_t[:, :, t0 : t0 + nt].rearrange("c p n -> p c n"),
        )

        scores = spsum.tile([128, 4 * R], F32, tag="sc")
        qsb = []
        for r in range(R):
            qp = qpsum.tile([128, NT], F32, tag="qp")
            nc.tensor.matmul(qp[:, 0:nt], w1sb[:, 0, :], ft[:, 0, 0:nt],
                             start=True, stop=False)
            nc.tensor.matmul(qp[:, 0:nt], w1sb[:, 1, :], ft[:, 1, 0:nt],
                             start=False, stop=False)
            nc.tensor.matmul(qp[:, 0:nt], w2sb[:, 0, :], tt[:, r, 0, 0:nt],
                             start=False, stop=False)
            nc.tensor.matmul(qp[:, 0:nt], w2sb[:, 1, :], tt[:, r, 1, 0:nt],
                             start=False, stop=True)

            q = qpool.tile([128, NT], BF16, tag="q")
            nc.scalar.activation(q[:, 0:nt], qp[:, 0:nt], ACTF.Tanh)
            qsb.append(q)

            for a, ns in blocks:
                nc.tensor.matmul(
                    scores[0:ns, a * R + r : a * R + r + 1],
                    q[:, a * PB : a * PB + ns],
                    msb[:, 0:1],
                    start=True, stop=True,
                )

        osb = opool.tile([128, 4, D], F32, tag="osb")
        for a, ns in blocks:
            sc = scores[0:ns, a * R : (a + 1) * R]
            negmax = small.tile([128, 1], F32, tag="negmax")
            nc.vector.tensor_reduce(negmax[0:ns], sc, AX.X, ALU.max, negate=True)
            expo = small.tile([128, R], F32, tag="expo")
            sume = small.tile([128, 1], F32, tag="sume")
            nc.scalar.activation(expo[0:ns], sc, ACTF.Exp,
                                 bias=negmax[0:ns], accum_out=sume[0:ns])
            inv = small.tile([128, 1], F32, tag="inv")
            nc.vector.reciprocal(inv[0:ns], sume[0:ns])

            fp = fpsum.tile([128, D], F32, tag="fp")
            for r in range(R):
                diag = small.tile([128, 128], BF16, tag="diag")
                nc.vector.tensor_scalar_mul(
                    diag[0:ns, 0:ns], eyesb[0:ns, 0:ns], expo[0:ns, r : r + 1]
                )
                nc.tensor.matmul(fp[0:ns, :], diag[0:ns, 0:ns],
                                 tn[0:ns, a, r, :],
                                 start=(r == 0), stop=(r == R - 1))
            nc.vector.tensor_scalar_mul(osb[0:ns, a, :], fp[0:ns, :],
                                        inv[0:ns, 0:1])

        nc.sync.dma_start(
            out=out[t0 : t0 + nt].rearrange("(a p) d -> p a d", p=p),
            in_=osb[0:p, 0:na, :],
        )


def build_program_pt(n_nodes=N_NODES, num_devices=BS, opts=None, time_reps=None):
    nc = bacc.Bacc(
        "TRN2", target_bir_lowering=False, debug=False, num_devices=num_devices
    )
    temb_n = nc.dram_tensor("temb_n", [n_nodes, R, D], BF16, kind="ExternalInput").ap()
    temb_t = nc.dram_tensor(
        "temb_t", [R, 2, 128, n_nodes], BF16, kind="ExternalInput"
    ).ap()
    feat_t = nc.dram_tensor("feat_t", [2, 128, n_nodes], BF16, kind="ExternalInput").ap()
    w1t = nc.dram_tensor("w1t", [2, 128, D2], BF16, kind="ExternalInput").ap()
    w2t = nc.dram_tensor("w2t", [2, 128, D2], BF16, kind="ExternalInput").ap()
    mcol = nc.dram_tensor("mcol", [D2, 1], BF16, kind="ExternalInput").ap()
    eye = nc.dram_tensor("eye", [128, 128], BF16, kind="ExternalInput").ap()
    out = nc.dram_tensor("out", [n_nodes, D], F32, kind="ExternalOutput").ap()

    with tile.TileContext(nc) as tc, ExitStack() as ctx:
        build_kernel_body_pt(
            ctx, tc, n_nodes,
            (temb_n, temb_t, feat_t, w1t, w2t, mcol, eye, out), opts=opts,
            time_reps=time_reps,
        )
    nc.compile()
    return nc


def make_pt_inputs(feature, temb):
    """Host-side bf16 cast + transpose for the pretransposed variant.
    feature: (bs, N, D) f32; temb: (bs, N, R, D) f32."""
    bf = ml_dtypes.bfloat16
    bs, n = feature.shape[0], feature.shape[1]
    temb_bf = temb.astype(bf)
    temb_n = temb_bf  # (bs, N, R, D)
    temb_t = np.ascontiguousarray(
        temb_bf.reshape(bs, n, R, 2, 128).transpose(0, 2, 3, 4, 1)
    )  # (bs, R, 2, 128, N)
    feat_t = np.ascontiguousarray(
        feature.astype(bf).reshape(bs, n, 2, 128).transpose(0, 2, 3, 1)
    )  # (bs, 2, 128, N)
    return temb_n, temb_t, feat_t


def build_program(n_nodes=N_NODES, num_devices=BS, opts=None, time_reps=None):
    nc = bacc.Bacc(
        "TRN2", target_bir_lowering=False, debug=False, num_devices=num_devices
    )
    feat = nc.dram_tensor("feature", [n_nodes, D], F32, kind="ExternalInput").ap()
    temb = nc.dram_tensor("temb", [n_nodes, R, D], F32, kind="ExternalInput").ap()
    w1t = nc.dram_tensor("w1t", [2, 128, D2], BF16, kind="ExternalInput").ap()
    w2t = nc.dram_tensor("w2t", [2, 128, D2], BF16, kind="ExternalInput").ap()
    mcol = nc.dram_tensor("mcol", [D2, 1], BF16, kind="ExternalInput").ap()
    eye = nc.dram_tensor("eye", [128, 128], BF16, kind="ExternalInput").ap()
    out = nc.dram_tensor("out", [n_nodes, D], F32, kind="ExternalOutput").ap()

    with tile.TileContext(nc) as tc, ExitStack() as ctx:
        build_kernel_body(
            ctx, tc, n_nodes, (feat, temb, w1t, w2t, mcol, eye, out), opts=opts,
            time_reps=time_reps,
        )
    nc.compile()
    return nc


def make_const_inputs(w1, w2, m):
    bf = ml_dtypes.bfloat16
    w1t = np.ascontiguousarray(w1.T.astype(bf)).reshape(2, 128, D2)
    w2t = np.ascontiguousarray(w2.T.astype(bf)).reshape(2, 128, D2)
    mcol = np.ascontiguousarray(m.reshape(D2, 1).astype(bf))
    eye = np.eye(128, dtype=bf)
    return w1t, w2t, mcol, eye


_cached_nc_pt = None


def kernel_pt(feature, type_aware_emb, w1, w2, m):
    """Pretransposed-variant entry: host casts/transposes, device computes."""
    from concourse.bass_utils import run_bass_kernel_spmd

    global _cached_nc_pt
    if _cached_nc_pt is None:
        _cached_nc_pt = build_program_pt()
    nc = _cached_nc_pt

    w1t, w2t, mcol, eye = make_const_inputs(
        np.asarray(w1, np.float32), np.asarray(w2, np.float32),
        np.asarray(m, np.float32),
    )
    feature = np.asarray(feature, np.float32)
    temb = np.asarray(type_aware_emb, np.float32)
    temb_n, temb_t, feat_t = make_pt_inputs(feature, temb)
    in_maps = [
        {
            "temb_n": temb_n[i],
            "temb_t": temb_t[i],
            "feat_t": feat_t[i],
            "w1t": w1t,
            "w2t": w2t,
            "mcol": mcol,
            "eye": eye,
        }
        for i in range(BS)
    ]
    res = run_bass_kernel_spmd(nc, in_maps, list(range(BS)))
    out = np.stack([np.asarray(res.results[i]["out"]) for i in range(BS)])
    return out.reshape(BS, N_NODES, 1, D).astype(np.float32)


_cached_nc = None


def kernel(feature, type_aware_emb, w1, w2, m, _trace=False, _tmpdir=None):
    from concourse.bass_utils import run_bass_kernel_spmd

    global _cached_nc
    if _cached_nc is None:
        _cached_nc = build_program()
    nc = _cached_nc

    w1t, w2t, mcol, eye = make_const_inputs(
        np.asarray(w1, np.float32), np.asarray(w2, np.float32),
        np.asarray(m, np.float32),
    )
    feature = np.asarray(feature, np.float32)
    temb = np.asarray(type_aware_emb, np.float32)
    in_maps = [
        {
            "feature": feature[i],
            "temb": temb[i],
            "w1t": w1t,
            "w2t": w2t,
            "mcol": mcol,
            "eye": eye,
        }
        for i in range(BS)
    ]
    res = run_bass_kernel_spmd(
        nc, in_maps, list(range(BS)), trace=_trace, tmpdir=_tmpdir
    )
    out = np.stack([np.asarray(res.results[i]["out"]) for i in range(BS)])
    if _trace:
        kernel.last_result = res
    return out.reshape(BS, N_NODES, 1, D).astype(np.float32)


def build_program_dmaonly(n_nodes=N_NODES, num_devices=BS, time_reps=None,
                          cast=True):
    """Loads+stores only — measures the DMA floor of the base access pattern."""
    nc = bacc.Bacc(
        "TRN2", target_bir_lowering=False, debug=False, num_devices=num_devices
    )
    feat = nc.dram_tensor("feature", [n_nodes, D], F32, kind="ExternalInput").ap()
    temb = nc.dram_tensor("temb", [n_nodes, R, D], F32, kind="ExternalInput").ap()
    w1t = nc.dram_tensor("w1t", [2, 128, D2], BF16, kind="ExternalInput").ap()
    w2t = nc.dram_tensor("w2t", [2, 128, D2], BF16, kind="ExternalInput").ap()
    mcol = nc.dram_tensor("mcol", [D2, 1], BF16, kind="ExternalInput").ap()
    eye = nc.dram_tensor("eye", [128, 128], BF16, kind="ExternalInput").ap()
    out = nc.dram_tensor("out", [n_nodes, D], F32, kind="ExternalOutput").ap()

    with tile.TileContext(nc) as tc, ExitStack() as ctx:
        fio = ctx.enter_context(tc.tile_pool(name="fio", bufs=3))
        tio = ctx.enter_context(tc.tile_pool(name="tio", bufs=3))
        opool = ctx.enter_context(tc.tile_pool(name="opool", bufs=3))
        const = ctx.enter_context(tc.tile_pool(name="const", bufs=1))
        osrc = const.tile([128, 4, D], F32)
        nc.vector.memset(osrc[:], 0.25)
        rep_cm = tc.For_i(0, time_reps, 1) if time_reps else None
        if rep_cm is not None:
            ctx.enter_context(rep_cm)
        for t0 in range(0, n_nodes, NT):
            nt = min(NT, n_nodes - t0)
            na = (nt + PB - 1) // PB
            p = min(PB, nt)
            dt_load = BF16 if cast else F32
            fn = fio.tile([128, 4, D], dt_load, tag="fn")
            tn = tio.tile([128, 4, R, D], dt_load, tag="tn")
            nc.gpsimd.dma_start(
                out=fn[0:p, 0:na, :],
                in_=feat[t0 : t0 + nt].rearrange("(a p) d -> p a d", p=p),
            )
            nc.gpsimd.dma_start(
                out=tn[0:p, 0:na, :, :],
                in_=temb[t0 : t0 + nt].rearrange("(a p) r d -> p a r d", p=p),
            )
            osb = opool.tile([128, 4, D], F32, tag="osb")
            nc.vector.tensor_copy(osb[:], osrc[:])
            nc.sync.dma_start(
                out=out[t0 : t0 + nt].rearrange("(a p) d -> p a d", p=p),
                in_=osb[0:p, 0:na, :],
            )
    nc.compile()
    return nc

